# revision 20
# baseline (speedup 1.0000x reference)
"""Trainium2 Bass kernel for nn_Classifier (topk_masking).

Computation (B=8192, D_IN=2048, D_BOT=1024, C=1000, SHARE=500, TEMP=0.05):
    feats = features @ W_b.T + b_b
    outputs1 = feats @ W1.T
    sm1 = masked_softmax(outputs1)          # row-local argmax-driven mask
    feat_oa = sm1 @ centroid
    features_aug = feats + feat_oa
    x = features_aug / max(||features_aug||_2, eps)
    outputs2 = (x @ W2.T) / TEMP
    softmax_outputs = softmax(outputs2)
returns (feats, features_aug, outputs1, outputs2, softmax_outputs)

Strategy: data-parallel over 8 NeuronCores (1024 batch rows each); all
weights replicated.  Matmuls run on the PE in bf16 with f32 PSUM
accumulation; the two matmuls feeding the argmax (MM1, MM2) use a 3-term
bf16 split (x = xh + xl;  x@W ~= xh@Wh + xh@Wl + xl@Wh) which makes the
row argmax bit-exact vs the f32 reference (verified: 0/8192 flips).
The mask+softmax stage is branchless:
    m_sh = max(o1[:, :500]);  m_pr = max(o1[:, 500:]);  rowmax = max(m_sh, m_pr)
    nonpriv = (m_pr <= m_sh)
    numer[:, :500]  = (o1_lo == rowmax)                  # 1 at argmax if shared
    numer[:, 500:]  = nonpriv * exp(o1_hi - rowmax) + (o1_hi == rowmax)
    sm1 = numer / sum(numer)
feat_oa uses  sm1T (PE-transposed) @ centroid  so the result lands in
row-major orientation.  outputs2 is computed as
    (featsH @ W2h.T + sm1 @ G) * (1/max(norm,eps)) / TEMP,   G = centroid @ W2.T
(G precomputed on host) which avoids transposing features_aug.
"""

import os
import sys

sys.path.insert(0, "/opt/trn_rl_repo")

import numpy as np
import ml_dtypes

B, D_IN, D_BOT, C = 8192, 2048, 1024, 1000
SHARE = 500
TEMP = 0.05
EPS = 1e-12
NCORES = 8
BC = B // NCORES          # rows per core
MT = BC // 128            # m-tiles per core
KT = D_IN // 128          # k-tiles over D_IN
NBT = D_BOT // 128        # tiles over D_BOT
CT = 8                    # c-tiles over padded C (1024)
CPAD = 1024

BF16 = ml_dtypes.bfloat16

_CACHE = {}
LAST_RESULT = None
REPEAT = 1      # bench knob: repeat the whole computation inside the kernel


def _bf16_hl(x):
    """Split f32 array into bf16 high + bf16 low parts (h + l ~= x)."""
    x = np.asarray(x, np.float32)
    h = x.astype(BF16)
    l = (x - h.astype(np.float32)).astype(BF16)
    return h, l


def _build():
    import concourse.bacc as bacc
    import concourse.tile as tile
    import concourse.mybir as mybir
    from concourse import masks
    from contextlib import ExitStack

    dt = mybir.dt
    Alu = mybir.AluOpType
    Act = mybir.ActivationFunctionType
    Ax = mybir.AxisListType

    nc = bacc.Bacc("TRN2", target_bir_lowering=False, debug=False,
                   num_devices=NCORES)

    # ---- DRAM I/O ----
    din = {}
    def inp(name, shape, dtype):
        din[name] = nc.dram_tensor(name, shape, dtype, kind="ExternalInput").ap()
        return din[name]

    fth_d = inp("fth", [128, MT * KT * 128], dt.bfloat16)   # featuresT high, [p, mt, t, m]
    ftl_d = inp("ftl", [128, MT * KT * 128], dt.bfloat16)
    wbh_d = inp("wbh", [128, KT * D_BOT], dt.bfloat16)      # W_b.T high, [p, t, nb]
    wbl_d = inp("wbl", [128, KT * D_BOT], dt.bfloat16)
    w1h_d = inp("w1h", [128, NBT * C], dt.bfloat16)         # W1.T high, [p, j, c]
    w1l_d = inp("w1l", [128, NBT * C], dt.bfloat16)
    w2h_d = inp("w2h", [128, NBT * C], dt.bfloat16)         # W2.T high, [p, j, c]
    cen_d = inp("cen", [128, CT * D_BOT], dt.bfloat16)      # centroid pad, [p, ct, nb]
    g_d = inp("g", [128, CT * C], dt.bfloat16)              # G = cent @ W2.T pad, [p, ct, c]
    bbh_d = inp("bbh", [1, D_BOT], dt.bfloat16)             # b_b high
    bbl_d = inp("bbl", [1, D_BOT], dt.bfloat16)

    feats_d = nc.dram_tensor("feats", [BC, D_BOT], dt.float32, kind="ExternalOutput").ap()
    fa_d = nc.dram_tensor("fa", [BC, D_BOT], dt.float32, kind="ExternalOutput").ap()
    o1_d = nc.dram_tensor("o1", [BC, C], dt.float32, kind="ExternalOutput").ap()
    o2_d = nc.dram_tensor("o2", [BC, C], dt.float32, kind="ExternalOutput").ap()
    sm_d = nc.dram_tensor("sm", [BC, C], dt.float32, kind="ExternalOutput").ap()

    with tile.TileContext(nc) as tc, ExitStack() as ctx:
        P = ctx.enter_context

        wpool = P(tc.tile_pool(name="weights", bufs=1))
        # resident weights
        wbh = wpool.tile([128, KT * D_BOT], dt.bfloat16)
        wbl = wpool.tile([128, KT * D_BOT], dt.bfloat16)
        w1h = wpool.tile([128, NBT * C], dt.bfloat16)
        w1l = wpool.tile([128, NBT * C], dt.bfloat16)
        w2h = wpool.tile([128, NBT * C], dt.bfloat16)
        cen = wpool.tile([128, CT * D_BOT], dt.bfloat16)
        g = wpool.tile([128, CT * C], dt.bfloat16)
        bbh = wpool.tile([1, D_BOT], dt.bfloat16)
        bbl = wpool.tile([1, D_BOT], dt.bfloat16)
        ones1 = wpool.tile([1, 128], dt.bfloat16)
        id32 = wpool.tile([128, 128], dt.float32)
        id16 = wpool.tile([128, 128], dt.bfloat16)

        # W_b chunked per k-tile so MM1 of the first m-tile can start as soon
        # as its first chunks land; weights used later (W1/W2/cent/G) are
        # queued afterwards so they don't delay the pipeline start.
        nc.sync.dma_start(wbh[:, 0:512], wbh_d[:, 0:512])
        nc.sync.dma_start(wbl[:, 0:512], wbl_d[:, 0:512])
        nc.sync.dma_start(wbh[:, 512:D_BOT], wbh_d[:, 512:D_BOT])
        nc.sync.dma_start(wbl[:, 512:D_BOT], wbl_d[:, 512:D_BOT])
        for t in range(1, KT):
            s = slice(t * D_BOT, (t + 1) * D_BOT)
            nc.sync.dma_start(wbh[:, s], wbh_d[:, s])
            nc.sync.dma_start(wbl[:, s], wbl_d[:, s])
        nc.sync.dma_start(bbh[:], bbh_d[:])
        nc.sync.dma_start(bbl[:], bbl_d[:])
        # W1 chunked per nb-tile (first MM2 is ~25us in)
        for j in range(NBT):
            s = slice(j * C, (j + 1) * C)
            nc.sync.dma_start(w1h[:, s], w1h_d[:, s])
            nc.sync.dma_start(w1l[:, s], w1l_d[:, s])
        nc.sync.dma_start(cen[:], cen_d[:])
        nc.sync.dma_start(w2h[:], w2h_d[:])
        nc.sync.dma_start(g[:], g_d[:])
        nc.vector.memset(ones1[:], 1.0)
        masks.make_identity(nc, id32[:])
        masks.make_identity(nc, id16[:])

        # pools (per-m-tile rotating tiles)
        pft = P(tc.tile_pool(name="ft", bufs=2))
        pfe = P(tc.tile_pool(name="feats", bufs=1))
        pth = P(tc.tile_pool(name="fTh", bufs=2))
        ptl = P(tc.tile_pool(name="fTl", bufs=2))
        pn1 = P(tc.tile_pool(name="sm1n", bufs=1))
        pmk = P(tc.tile_pool(name="maskt", bufs=1))
        ps1 = P(tc.tile_pool(name="sm1b", bufs=1))
        ps1t = P(tc.tile_pool(name="sm1T", bufs=1))
        pfa = P(tc.tile_pool(name="fa", bufs=1))
        pe2 = P(tc.tile_pool(name="e2", bufs=1))
        psm = P(tc.tile_pool(name="sm2", bufs=1))
        po1sb = P(tc.tile_pool(name="o1sb", bufs=1))
        po2sb = P(tc.tile_pool(name="o2sb", bufs=1))
        pst = P(tc.tile_pool(name="stats", bufs=1))

        # Separate PSUM rings so MM1(i+1)'s slots recycle early (feats copy)
        # instead of waiting on the previous tile's mask stage.
        psum_f = P(tc.tile_pool(name="psf", bufs=3, space="PSUM"))   # f0,f1,oa0,oa1
        psum_o = P(tc.tile_pool(name="pso", bufs=2, space="PSUM"))   # o1a,o1b,p20,p21
        psumt = P(tc.tile_pool(name="pt", bufs=1, space="PSUM"))

        for rep in range(REPEAT):
          for mt in range(MT):
            rows = slice(mt * 128, (mt + 1) * 128)

            # --- load featuresT tiles for this m-tile ---
            fth = pft.tile([128, KT * 128], dt.bfloat16, tag="fth")
            ftl = pft.tile([128, KT * 128], dt.bfloat16, tag="ftl")
            nc.scalar.dma_start(fth[:], fth_d[:, mt * KT * 128:(mt + 1) * KT * 128])
            nc.scalar.dma_start(ftl[:], ftl_d[:, mt * KT * 128:(mt + 1) * KT * 128])

            # --- MM1: feats[m, nb] (3-term bf16 split), two 512-halves ---
            pf = []
            for h in range(2):
                pfh = psum_f.tile([128, 512], dt.float32, tag="psf")
                pf.append(pfh)
            for t in range(KT):
                lt_h = fth[:, t * 128:(t + 1) * 128]
                lt_l = ftl[:, t * 128:(t + 1) * 128]
                for h in range(2):
                    rs_h = wbh[:, t * D_BOT + h * 512: t * D_BOT + h * 512 + 512]
                    rs_l = wbl[:, t * D_BOT + h * 512: t * D_BOT + h * 512 + 512]
                    nc.tensor.matmul(pf[h][:], lt_h, rs_h, start=(t == 0), stop=False)
                    nc.tensor.matmul(pf[h][:], lt_h, rs_l, start=False, stop=False)
                    nc.tensor.matmul(pf[h][:], lt_l, rs_h, start=False, stop=False)
            # b_b via K=1 matmuls against a ones row
            for h in range(2):
                nc.tensor.matmul(pf[h][:], ones1[:1, :], bbh[:1, h * 512:h * 512 + 512],
                                 start=False, stop=False)
                nc.tensor.matmul(pf[h][:], ones1[:1, :], bbl[:1, h * 512:h * 512 + 512],
                                 start=False, stop=True)

            feats_sb = pfe.tile([128, D_BOT], dt.float32, tag="feats")
            nc.scalar.copy(feats_sb[:, 0:512], pf[0][:])
            nc.scalar.copy(feats_sb[:, 512:1024], pf[1][:])
            nc.scalar.dma_start(feats_d[rows, :], feats_sb[:])

            # --- transpose feats -> featsT (h/l split out of psum) ---
            fTh = pth.tile([128, D_BOT], dt.bfloat16, tag="fTh")
            fTl = ptl.tile([128, D_BOT], dt.bfloat16, tag="fTl")
            for j in range(NBT):
                ptj = psumt.tile([128, 128], dt.float32, tag="pt")
                nc.tensor.transpose(ptj[:], feats_sb[:, j * 128:(j + 1) * 128], id32[:])
                js = slice(j * 128, (j + 1) * 128)
                nc.scalar.copy(fTh[:, js], ptj[:])          # rounds f32 -> bf16
                nc.vector.scalar_tensor_tensor(
                    fTl[:, js], ptj[:], 1.0, fTh[:, js],
                    op0=Alu.mult, op1=Alu.subtract)
            # --- MM2: o1[m, c] (3-term bf16 split), halves of 500 ---
            po1 = []
            for h in range(2):
                p1h = psum_o.tile([128, 512], dt.float32, tag="pso")
                for j in range(NBT):
                    lt_h = fTh[:, j * 128:(j + 1) * 128]
                    lt_l = fTl[:, j * 128:(j + 1) * 128]
                    rs_h = w1h[:, j * C + h * 500: j * C + h * 500 + 500]
                    rs_l = w1l[:, j * C + h * 500: j * C + h * 500 + 500]
                    nc.tensor.matmul(p1h[:, :500], lt_h, rs_h, start=(j == 0), stop=False)
                    nc.tensor.matmul(p1h[:, :500], lt_h, rs_l, start=False, stop=False)
                    nc.tensor.matmul(p1h[:, :500], lt_l, rs_h, start=False,
                                     stop=(j == NBT - 1))
                po1.append(p1h)
            o1sb = po1sb.tile([128, C], dt.float32, tag="o1sb")
            nc.scalar.copy(o1sb[:, 0:500], po1[0][:, :500])
            nc.scalar.copy(o1sb[:, 500:1000], po1[1][:, :500])
            nc.scalar.dma_start(o1_d[rows, :], o1sb[:])

            # --- mask + softmax1 (branchless) ---
            m_sh = pst.tile([128, 1], dt.float32, tag="msh")
            m_pr = pst.tile([128, 1], dt.float32, tag="mpr")
            nc.vector.tensor_reduce(m_sh[:], po1[0][:, :500], Ax.X, Alu.max)
            nc.vector.tensor_reduce(m_pr[:], po1[1][:, :500], Ax.X, Alu.max)
            rowmax = pst.tile([128, 1], dt.float32, tag="rmx")
            nc.vector.tensor_tensor(rowmax[:], m_sh[:], m_pr[:], op=Alu.max)
            negmax = pst.tile([128, 1], dt.float32, tag="ngm")
            nc.vector.tensor_scalar_mul(negmax[:], rowmax[:], -1.0)
            nonpriv = pst.tile([128, 1], dt.float32, tag="npv")
            nc.vector.tensor_tensor(nonpriv[:], m_pr[:], m_sh[:], op=Alu.is_le)

            sm1n = pn1.tile([128, CPAD], dt.bfloat16, tag="sm1n")
            s_lo = pst.tile([128, 1], dt.float32, tag="slo")
            s_hi = pst.tile([128, 1], dt.float32, tag="shi")
            # lo half: numerator = (o1 == rowmax); exp(o1-rowmax)=1 there
            nc.vector.tensor_scalar(sm1n[:, 0:500], po1[0][:, :500], rowmax[:], None,
                                    op0=Alu.is_equal, op1=Alu.add, accum_out=s_lo[:])
            # hi half: nonpriv * exp(o1 - rowmax) + (o1 == rowmax)
            e_hi = pmk.tile([128, 512], dt.float32, tag="ehi")
            eq_hi = pmk.tile([128, 512], dt.float32, tag="eqh")
            nc.scalar.activation(e_hi[:, :500], po1[1][:, :500], Act.Exp,
                                 bias=negmax[:], scale=1.0)
            nc.vector.tensor_scalar(eq_hi[:, :500], po1[1][:, :500], rowmax[:], None,
                                    op0=Alu.is_equal)
            nc.vector.scalar_tensor_tensor(
                sm1n[:, 500:1000], e_hi[:, :500], nonpriv[:], eq_hi[:, :500],
                op0=Alu.mult, op1=Alu.add, accum_out=s_hi[:])
            nc.vector.memset(sm1n[:, 1000:CPAD], 0.0)

            s_all = pst.tile([128, 1], dt.float32, tag="sal")
            nc.vector.tensor_tensor(s_all[:], s_lo[:], s_hi[:], op=Alu.add)
            rs1 = pst.tile([128, 1], dt.float32, tag="rs1")
            nc.vector.reciprocal(rs1[:], s_all[:])
            sm1b = ps1.tile([128, CPAD], dt.bfloat16, tag="sm1b")
            nc.vector.tensor_scalar(sm1b[:], sm1n[:], rs1[:], None, op0=Alu.mult)

            # --- transpose sm1 -> sm1T ---
            sm1T = ps1t.tile([128, CPAD], dt.bfloat16, tag="sm1T")
            for j in range(CT):
                ptj = psumt.tile([128, 128], dt.bfloat16, tag="pt")
                nc.tensor.transpose(ptj[:], sm1b[:, j * 128:(j + 1) * 128], id16[:])
                nc.scalar.copy(sm1T[:, j * 128:(j + 1) * 128], ptj[:])

            # --- MM3: feat_oa[m, nb] = sm1 @ centroid ---
            poa = []
            for h in range(2):
                pah = psum_f.tile([128, 512], dt.float32, tag="psf")
                for ct in range(CT):
                    nc.tensor.matmul(
                        pah[:], sm1T[:, ct * 128:(ct + 1) * 128],
                        cen[:, ct * D_BOT + h * 512: ct * D_BOT + h * 512 + 512],
                        start=(ct == 0), stop=(ct == CT - 1))
                poa.append(pah)

            # --- features_aug = feats + feat_oa;  row sum of squares ---
            fa_sb = pfa.tile([128, D_BOT], dt.float32, tag="fa")
            nc.vector.tensor_tensor(fa_sb[:, 0:512], feats_sb[:, 0:512], poa[0][:], op=Alu.add)
            nc.vector.tensor_tensor(fa_sb[:, 512:1024], feats_sb[:, 512:1024], poa[1][:], op=Alu.add)
            nc.scalar.dma_start(fa_d[rows, :], fa_sb[:])

            e2 = pe2.tile([128, CPAD], dt.float32, tag="e2")  # reused as sq scratch
            ss_lo = pst.tile([128, 1], dt.float32, tag="sql")
            ss_hi = pst.tile([128, 1], dt.float32, tag="sqh")
            nc.scalar.activation(e2[:, 0:512], fa_sb[:, 0:512], Act.Square,
                                 accum_out=ss_lo[:])
            nc.scalar.activation(e2[:, 512:1024], fa_sb[:, 512:1024], Act.Square,
                                 accum_out=ss_hi[:])
            ss = pst.tile([128, 1], dt.float32, tag="ssa")
            nc.vector.tensor_tensor(ss[:], ss_lo[:], ss_hi[:], op=Alu.add)
            nrm = pst.tile([128, 1], dt.float32, tag="nrm")
            nc.scalar.activation(nrm[:], ss[:], Act.Sqrt)
            nc.vector.tensor_scalar_max(nrm[:], nrm[:], EPS)
            rn = pst.tile([128, 1], dt.float32, tag="rn")
            nc.vector.reciprocal(rn[:], nrm[:])
            scl = pst.tile([128, 1], dt.float32, tag="scl")
            nc.vector.tensor_scalar_mul(scl[:], rn[:], 1.0 / TEMP)

            # --- outputs2 raw: featsH @ W2h.T + sm1 @ G ---
            # W2-term first for both halves (depends only on fTh, fills the
            # PE while the mask stage runs), then the G-term (needs sm1T).
            po2 = []
            for h in range(2):
                p2h = psum_o.tile([128, 512], dt.float32, tag="pso")
                for j in range(NBT):
                    nc.tensor.matmul(
                        p2h[:, :500], fTh[:, j * 128:(j + 1) * 128],
                        w2h[:, j * C + h * 500: j * C + h * 500 + 500],
                        start=(j == 0), stop=False)
                po2.append(p2h)
            for h in range(2):
                for ct in range(CT):
                    nc.tensor.matmul(
                        po2[h][:, :500], sm1T[:, ct * 128:(ct + 1) * 128],
                        g[:, ct * C + h * 500: ct * C + h * 500 + 500],
                        start=False, stop=(ct == CT - 1))

            # rowmax of raw o2 (scale-invariant since scl > 0)
            r2_lo = pst.tile([128, 1], dt.float32, tag="r2l")
            r2_hi = pst.tile([128, 1], dt.float32, tag="r2h")
            nc.vector.tensor_reduce(r2_lo[:], po2[0][:, :500], Ax.X, Alu.max)
            nc.vector.tensor_reduce(r2_hi[:], po2[1][:, :500], Ax.X, Alu.max)
            r2 = pst.tile([128, 1], dt.float32, tag="r2")
            nc.vector.tensor_tensor(r2[:], r2_lo[:], r2_hi[:], op=Alu.max)
            # bias for exp: -(r2 * scl)
            b2 = pst.tile([128, 1], dt.float32, tag="b2")
            nc.vector.scalar_tensor_tensor(b2[:], r2[:], -1.0, scl[:],
                                           op0=Alu.mult, op1=Alu.mult)
            # scale o2 out of psum into SBUF (fused scale+copy), write out
            o2sb = po2sb.tile([128, C], dt.float32, tag="o2sb")
            nc.vector.tensor_scalar(o2sb[:, 0:500], po2[0][:, :500], scl[:], None, op0=Alu.mult)
            nc.vector.tensor_scalar(o2sb[:, 500:1000], po2[1][:, :500], scl[:], None, op0=Alu.mult)
            nc.scalar.dma_start(o2_d[rows, :], o2sb[:])

            # --- softmax2 ---
            s2_lo = pst.tile([128, 1], dt.float32, tag="s2l")
            s2_hi = pst.tile([128, 1], dt.float32, tag="s2h")
            nc.scalar.activation(e2[:, 0:500], o2sb[:, 0:500], Act.Exp,
                                 bias=b2[:], scale=1.0, accum_out=s2_lo[:])
            nc.scalar.activation(e2[:, 500:1000], o2sb[:, 500:1000], Act.Exp,
                                 bias=b2[:], scale=1.0, accum_out=s2_hi[:])
            s2 = pst.tile([128, 1], dt.float32, tag="s2")
            nc.vector.tensor_tensor(s2[:], s2_lo[:], s2_hi[:], op=Alu.add)
            rs2 = pst.tile([128, 1], dt.float32, tag="rs2")
            nc.vector.reciprocal(rs2[:], s2[:])
            sm2 = psm.tile([128, C], dt.float32, tag="sm2")
            nc.vector.tensor_scalar(sm2[:], e2[:, 0:1000], rs2[:], None, op0=Alu.mult)
            nc.scalar.dma_start(sm_d[rows, :], sm2[:])

    nc.compile()
    return nc


def _prep_inputs(features, W_b, b_b, W1, W2, centroid):
    """Host-side prep: per-core input maps with pre-transposed/split weights."""
    features = np.asarray(features, np.float32)
    W_b = np.asarray(W_b, np.float32)
    b_b = np.asarray(b_b, np.float32)
    W1 = np.asarray(W1, np.float32)
    W2 = np.asarray(W2, np.float32)
    centroid = np.asarray(centroid, np.float32)

    # W_b.T -> [p, t, nb] layout
    wbt = np.ascontiguousarray(W_b.T).reshape(KT, 128, D_BOT).transpose(1, 0, 2)
    wbh, wbl = _bf16_hl(wbt.reshape(128, KT * D_BOT))
    # W1.T -> [p, j, c]
    w1t = np.ascontiguousarray(W1.T).reshape(NBT, 128, C).transpose(1, 0, 2)
    w1h, w1l = _bf16_hl(w1t.reshape(128, NBT * C))
    # W2.T -> [p, j, c] (high part only)
    w2t = np.ascontiguousarray(W2.T).reshape(NBT, 128, C).transpose(1, 0, 2)
    w2h = w2t.reshape(128, NBT * C).astype(BF16)
    # centroid padded to 1024 rows -> [p, ct, nb]
    cpad = np.zeros((CPAD, D_BOT), np.float32)
    cpad[:C] = centroid
    cen = cpad.reshape(CT, 128, D_BOT).transpose(1, 0, 2).reshape(128, CT * D_BOT).astype(BF16)
    # G = centroid @ W2.T, padded rows -> [p, ct, c]
    G = centroid @ W2.T
    gpad = np.zeros((CPAD, C), np.float32)
    gpad[:C] = G
    g = gpad.reshape(CT, 128, C).transpose(1, 0, 2).reshape(128, CT * C).astype(BF16)
    bbh, bbl = _bf16_hl(b_b.reshape(1, D_BOT))

    shared = dict(wbh=wbh, wbl=wbl, w1h=w1h, w1l=w1l, w2h=w2h, cen=cen, g=g,
                  bbh=bbh, bbl=bbl)

    fh, fl = _bf16_hl(features)
    in_maps = []
    for c in range(NCORES):
        m = dict(shared)
        for name, arr in (("fth", fh), ("ftl", fl)):
            blk = arr[c * BC:(c + 1) * BC]                      # [1024, 2048]
            # [mt, m, t, p] -> [p, mt, t, m]
            ft = blk.reshape(MT, 128, KT, 128).transpose(3, 0, 2, 1)
            m[name] = np.ascontiguousarray(ft).reshape(128, MT * KT * 128)
        in_maps.append(m)
    return in_maps


def kernel(features, W_b, b_b, W1, W2, centroid):
    global LAST_RESULT
    from concourse.bass_utils import run_bass_kernel_spmd

    if "nc" not in _CACHE:
        _CACHE["nc"] = _build()
    nc = _CACHE["nc"]

    in_maps = _prep_inputs(features, W_b, b_b, W1, W2, centroid)
    # NTFF tracing is unavailable in this container (antenv.axon_hooks is a
    # stub) and would crash run_bass_kernel_spmd if BASS_TRACE leaks in.
    os.environ["BASS_NEVER_TRACE"] = "1"
    res = run_bass_kernel_spmd(nc, in_maps, list(range(NCORES)), trace=False)
    LAST_RESULT = res

    feats = np.concatenate([r["feats"] for r in res.results], axis=0)
    fa = np.concatenate([r["fa"] for r in res.results], axis=0)
    o1 = np.concatenate([r["o1"] for r in res.results], axis=0)
    o2 = np.concatenate([r["o2"] for r in res.results], axis=0)
    sm = np.concatenate([r["sm"] for r in res.results], axis=0)
    return feats, fa, o1, o2, sm


# revision 21
# speedup vs baseline: 1.0194x; 1.0194x over previous
"""Trainium2 Bass kernel for nn_Classifier (topk_masking).

Computation (B=8192, D_IN=2048, D_BOT=1024, C=1000, SHARE=500, TEMP=0.05):
    feats = features @ W_b.T + b_b
    outputs1 = feats @ W1.T
    sm1 = masked_softmax(outputs1)          # row-local argmax-driven mask
    feat_oa = sm1 @ centroid
    features_aug = feats + feat_oa
    x = features_aug / max(||features_aug||_2, eps)
    outputs2 = (x @ W2.T) / TEMP
    softmax_outputs = softmax(outputs2)
returns (feats, features_aug, outputs1, outputs2, softmax_outputs)

Strategy: data-parallel over 8 NeuronCores (1024 batch rows each); all
weights replicated.  Matmuls run on the PE in bf16 with f32 PSUM
accumulation; the two matmuls feeding the argmax (MM1, MM2) use a 3-term
bf16 split (x = xh + xl;  x@W ~= xh@Wh + xh@Wl + xl@Wh) which makes the
row argmax bit-exact vs the f32 reference (verified: 0/8192 flips).
The mask+softmax stage is branchless:
    m_sh = max(o1[:, :500]);  m_pr = max(o1[:, 500:]);  rowmax = max(m_sh, m_pr)
    nonpriv = (m_pr <= m_sh)
    numer[:, :500]  = (o1_lo == rowmax)                  # 1 at argmax if shared
    numer[:, 500:]  = nonpriv * exp(o1_hi - rowmax) + (o1_hi == rowmax)
    sm1 = numer / sum(numer)
feat_oa uses  sm1T (PE-transposed) @ centroid  so the result lands in
row-major orientation.  outputs2 is computed as
    (featsH @ W2h.T + sm1 @ G) * (1/max(norm,eps)) / TEMP,   G = centroid @ W2.T
(G precomputed on host) which avoids transposing features_aug.
"""

import os
import sys

sys.path.insert(0, "/opt/trn_rl_repo")

import numpy as np
import ml_dtypes

B, D_IN, D_BOT, C = 8192, 2048, 1024, 1000
SHARE = 500
TEMP = 0.05
EPS = 1e-12
NCORES = 8
BC = B // NCORES          # rows per core
MT = BC // 128            # m-tiles per core
KT = D_IN // 128          # k-tiles over D_IN
NBT = D_BOT // 128        # tiles over D_BOT
CT = 8                    # c-tiles over padded C (1024)
CPAD = 1024

BF16 = ml_dtypes.bfloat16

_CACHE = {}
LAST_RESULT = None
REPEAT = 1      # bench knob: repeat the whole computation inside the kernel


def _bf16_hl(x):
    """Split f32 array into bf16 high + bf16 low parts (h + l ~= x)."""
    x = np.asarray(x, np.float32)
    h = x.astype(BF16)
    l = (x - h.astype(np.float32)).astype(BF16)
    return h, l


def _build(use_bias=True):
    import concourse.bacc as bacc
    import concourse.tile as tile
    import concourse.mybir as mybir
    from concourse import masks
    from contextlib import ExitStack

    dt = mybir.dt
    Alu = mybir.AluOpType
    Act = mybir.ActivationFunctionType
    Ax = mybir.AxisListType

    nc = bacc.Bacc("TRN2", target_bir_lowering=False, debug=False,
                   num_devices=NCORES)

    # ---- DRAM I/O ----
    din = {}
    def inp(name, shape, dtype):
        din[name] = nc.dram_tensor(name, shape, dtype, kind="ExternalInput").ap()
        return din[name]

    fth_d = inp("fth", [128, MT * KT * 128], dt.bfloat16)   # featuresT high, [p, mt, t, m]
    ftl_d = inp("ftl", [128, MT * KT * 128], dt.bfloat16)
    wbh_d = inp("wbh", [128, KT * D_BOT], dt.bfloat16)      # W_b.T high, [p, t, nb]
    wbl_d = inp("wbl", [128, KT * D_BOT], dt.bfloat16)
    w1h_d = inp("w1h", [128, NBT * C], dt.bfloat16)         # W1.T high, [p, j, c]
    w1l_d = inp("w1l", [128, NBT * C], dt.bfloat16)
    w2h_d = inp("w2h", [128, NBT * C], dt.bfloat16)         # W2.T high, [p, j, c]
    cen_d = inp("cen", [128, CT * D_BOT], dt.bfloat16)      # centroid pad, [p, ct, nb]
    g_d = inp("g", [128, CT * C], dt.bfloat16)              # G = cent @ W2.T pad, [p, ct, c]
    bbh_d = inp("bbh", [1, D_BOT], dt.bfloat16)             # b_b high
    bbl_d = inp("bbl", [1, D_BOT], dt.bfloat16)

    feats_d = nc.dram_tensor("feats", [BC, D_BOT], dt.float32, kind="ExternalOutput").ap()
    fa_d = nc.dram_tensor("fa", [BC, D_BOT], dt.float32, kind="ExternalOutput").ap()
    o1_d = nc.dram_tensor("o1", [BC, C], dt.float32, kind="ExternalOutput").ap()
    o2_d = nc.dram_tensor("o2", [BC, C], dt.float32, kind="ExternalOutput").ap()
    sm_d = nc.dram_tensor("sm", [BC, C], dt.float32, kind="ExternalOutput").ap()

    with tile.TileContext(nc) as tc, ExitStack() as ctx:
        P = ctx.enter_context

        wpool = P(tc.tile_pool(name="weights", bufs=1))
        # resident weights
        wbh = wpool.tile([128, KT * D_BOT], dt.bfloat16)
        wbl = wpool.tile([128, KT * D_BOT], dt.bfloat16)
        w1h = wpool.tile([128, NBT * C], dt.bfloat16)
        w1l = wpool.tile([128, NBT * C], dt.bfloat16)
        w2h = wpool.tile([128, NBT * C], dt.bfloat16)
        cen = wpool.tile([128, CT * D_BOT], dt.bfloat16)
        g = wpool.tile([128, CT * C], dt.bfloat16)
        bbh = wpool.tile([1, D_BOT], dt.bfloat16)
        bbl = wpool.tile([1, D_BOT], dt.bfloat16)
        ones1 = wpool.tile([1, 128], dt.bfloat16)
        id32 = wpool.tile([128, 128], dt.float32)
        id16 = wpool.tile([128, 128], dt.bfloat16)

        # W_b chunked per k-tile so MM1 of the first m-tile can start as soon
        # as its first chunks land; weights used later (W1/W2/cent/G) are
        # queued afterwards so they don't delay the pipeline start.
        nc.sync.dma_start(wbh[:, 0:512], wbh_d[:, 0:512])
        nc.sync.dma_start(wbl[:, 0:512], wbl_d[:, 0:512])
        nc.sync.dma_start(wbh[:, 512:D_BOT], wbh_d[:, 512:D_BOT])
        nc.sync.dma_start(wbl[:, 512:D_BOT], wbl_d[:, 512:D_BOT])
        for t in range(1, KT):
            s = slice(t * D_BOT, (t + 1) * D_BOT)
            nc.sync.dma_start(wbh[:, s], wbh_d[:, s])
            nc.sync.dma_start(wbl[:, s], wbl_d[:, s])
        nc.sync.dma_start(bbh[:], bbh_d[:])
        nc.sync.dma_start(bbl[:], bbl_d[:])
        # W1 chunked per nb-tile (first MM2 is ~25us in)
        for j in range(NBT):
            s = slice(j * C, (j + 1) * C)
            nc.sync.dma_start(w1h[:, s], w1h_d[:, s])
            nc.sync.dma_start(w1l[:, s], w1l_d[:, s])
        nc.sync.dma_start(cen[:], cen_d[:])
        nc.sync.dma_start(w2h[:], w2h_d[:])
        nc.sync.dma_start(g[:], g_d[:])
        nc.vector.memset(ones1[:], 1.0)
        masks.make_identity(nc, id32[:])
        masks.make_identity(nc, id16[:])

        # pools (per-m-tile rotating tiles)
        pft = P(tc.tile_pool(name="ft", bufs=2))
        pfe = P(tc.tile_pool(name="feats", bufs=1))
        pth = P(tc.tile_pool(name="fTh", bufs=2))
        ptl = P(tc.tile_pool(name="fTl", bufs=2))
        pn1 = P(tc.tile_pool(name="sm1n", bufs=1))
        pmk = P(tc.tile_pool(name="maskt", bufs=1))
        ps1 = P(tc.tile_pool(name="sm1b", bufs=1))
        ps1t = P(tc.tile_pool(name="sm1T", bufs=1))
        pfa = P(tc.tile_pool(name="fa", bufs=1))
        pe2 = P(tc.tile_pool(name="e2", bufs=1))
        psm = P(tc.tile_pool(name="sm2", bufs=1))
        po1sb = P(tc.tile_pool(name="o1sb", bufs=1))
        po2sb = P(tc.tile_pool(name="o2sb", bufs=1))
        pst = P(tc.tile_pool(name="stats", bufs=1))

        # Separate PSUM rings so MM1(i+1)'s slots recycle early (feats copy)
        # instead of waiting on the previous tile's mask stage.
        psum_f = P(tc.tile_pool(name="psf", bufs=3, space="PSUM"))   # f0,f1,oa0,oa1
        psum_o = P(tc.tile_pool(name="pso", bufs=2, space="PSUM"))   # o1a,o1b,p20,p21
        psumt = P(tc.tile_pool(name="pt", bufs=1, space="PSUM"))

        for rep in range(REPEAT):
          for mt in range(MT):
            rows = slice(mt * 128, (mt + 1) * 128)

            # --- load featuresT tiles for this m-tile ---
            fth = pft.tile([128, KT * 128], dt.bfloat16, tag="fth")
            ftl = pft.tile([128, KT * 128], dt.bfloat16, tag="ftl")
            nc.scalar.dma_start(fth[:], fth_d[:, mt * KT * 128:(mt + 1) * KT * 128])
            nc.scalar.dma_start(ftl[:], ftl_d[:, mt * KT * 128:(mt + 1) * KT * 128])

            # --- MM1: feats[m, nb] (3-term bf16 split), two 512-halves ---
            pf = []
            for h in range(2):
                pfh = psum_f.tile([128, 512], dt.float32, tag="psf")
                pf.append(pfh)
            for t in range(KT):
                lt_h = fth[:, t * 128:(t + 1) * 128]
                lt_l = ftl[:, t * 128:(t + 1) * 128]
                last = (not use_bias) and (t == KT - 1)
                for h in range(2):
                    rs_h = wbh[:, t * D_BOT + h * 512: t * D_BOT + h * 512 + 512]
                    rs_l = wbl[:, t * D_BOT + h * 512: t * D_BOT + h * 512 + 512]
                    nc.tensor.matmul(pf[h][:], lt_h, rs_h, start=(t == 0), stop=False)
                    nc.tensor.matmul(pf[h][:], lt_h, rs_l, start=False, stop=False)
                    nc.tensor.matmul(pf[h][:], lt_l, rs_h, start=False, stop=last)
            # b_b via K=1 matmuls against a ones row (skipped when b_b == 0:
            # even a K=1 matmul streams the full N=512 cycles)
            if use_bias:
                for h in range(2):
                    nc.tensor.matmul(pf[h][:], ones1[:1, :],
                                     bbh[:1, h * 512:h * 512 + 512],
                                     start=False, stop=False)
                    nc.tensor.matmul(pf[h][:], ones1[:1, :],
                                     bbl[:1, h * 512:h * 512 + 512],
                                     start=False, stop=True)

            feats_sb = pfe.tile([128, D_BOT], dt.float32, tag="feats")
            nc.scalar.copy(feats_sb[:, 0:512], pf[0][:])
            nc.scalar.copy(feats_sb[:, 512:1024], pf[1][:])
            nc.scalar.dma_start(feats_d[rows, :], feats_sb[:])

            # --- transpose feats -> featsT (h/l split out of psum) ---
            fTh = pth.tile([128, D_BOT], dt.bfloat16, tag="fTh")
            fTl = ptl.tile([128, D_BOT], dt.bfloat16, tag="fTl")
            for j in range(NBT):
                ptj = psumt.tile([128, 128], dt.float32, tag="pt")
                nc.tensor.transpose(ptj[:], feats_sb[:, j * 128:(j + 1) * 128], id32[:])
                js = slice(j * 128, (j + 1) * 128)
                nc.scalar.copy(fTh[:, js], ptj[:])          # rounds f32 -> bf16
                nc.vector.scalar_tensor_tensor(
                    fTl[:, js], ptj[:], 1.0, fTh[:, js],
                    op0=Alu.mult, op1=Alu.subtract)
            # --- MM2: o1[m, c] (3-term bf16 split), halves of 500 ---
            po1 = []
            for h in range(2):
                p1h = psum_o.tile([128, 512], dt.float32, tag="pso")
                for j in range(NBT):
                    lt_h = fTh[:, j * 128:(j + 1) * 128]
                    lt_l = fTl[:, j * 128:(j + 1) * 128]
                    rs_h = w1h[:, j * C + h * 500: j * C + h * 500 + 500]
                    rs_l = w1l[:, j * C + h * 500: j * C + h * 500 + 500]
                    nc.tensor.matmul(p1h[:, :500], lt_h, rs_h, start=(j == 0), stop=False)
                    nc.tensor.matmul(p1h[:, :500], lt_h, rs_l, start=False, stop=False)
                    nc.tensor.matmul(p1h[:, :500], lt_l, rs_h, start=False,
                                     stop=(j == NBT - 1))
                po1.append(p1h)
            o1sb = po1sb.tile([128, C], dt.float32, tag="o1sb")
            nc.scalar.copy(o1sb[:, 0:500], po1[0][:, :500])
            nc.scalar.copy(o1sb[:, 500:1000], po1[1][:, :500])
            nc.scalar.dma_start(o1_d[rows, :], o1sb[:])

            # --- mask + softmax1 (branchless) ---
            m_sh = pst.tile([128, 1], dt.float32, tag="msh")
            m_pr = pst.tile([128, 1], dt.float32, tag="mpr")
            nc.vector.tensor_reduce(m_sh[:], po1[0][:, :500], Ax.X, Alu.max)
            nc.vector.tensor_reduce(m_pr[:], po1[1][:, :500], Ax.X, Alu.max)
            rowmax = pst.tile([128, 1], dt.float32, tag="rmx")
            nc.vector.tensor_tensor(rowmax[:], m_sh[:], m_pr[:], op=Alu.max)
            negmax = pst.tile([128, 1], dt.float32, tag="ngm")
            nc.vector.tensor_scalar_mul(negmax[:], rowmax[:], -1.0)
            nonpriv = pst.tile([128, 1], dt.float32, tag="npv")
            nc.vector.tensor_tensor(nonpriv[:], m_pr[:], m_sh[:], op=Alu.is_le)

            sm1n = pn1.tile([128, CPAD], dt.bfloat16, tag="sm1n")
            s_lo = pst.tile([128, 1], dt.float32, tag="slo")
            s_hi = pst.tile([128, 1], dt.float32, tag="shi")
            # lo half: numerator = (o1 == rowmax); exp(o1-rowmax)=1 there
            nc.vector.tensor_scalar(sm1n[:, 0:500], po1[0][:, :500], rowmax[:], None,
                                    op0=Alu.is_equal, op1=Alu.add, accum_out=s_lo[:])
            # hi half: nonpriv * exp(o1 - rowmax) + (o1 == rowmax)
            e_hi = pmk.tile([128, 512], dt.float32, tag="ehi")
            eq_hi = pmk.tile([128, 512], dt.float32, tag="eqh")
            nc.scalar.activation(e_hi[:, :500], po1[1][:, :500], Act.Exp,
                                 bias=negmax[:], scale=1.0)
            nc.vector.tensor_scalar(eq_hi[:, :500], po1[1][:, :500], rowmax[:], None,
                                    op0=Alu.is_equal)
            nc.vector.scalar_tensor_tensor(
                sm1n[:, 500:1000], e_hi[:, :500], nonpriv[:], eq_hi[:, :500],
                op0=Alu.mult, op1=Alu.add, accum_out=s_hi[:])
            nc.vector.memset(sm1n[:, 1000:CPAD], 0.0)

            s_all = pst.tile([128, 1], dt.float32, tag="sal")
            nc.vector.tensor_tensor(s_all[:], s_lo[:], s_hi[:], op=Alu.add)
            rs1 = pst.tile([128, 1], dt.float32, tag="rs1")
            nc.vector.reciprocal(rs1[:], s_all[:])
            sm1b = ps1.tile([128, CPAD], dt.bfloat16, tag="sm1b")
            nc.vector.tensor_scalar(sm1b[:], sm1n[:], rs1[:], None, op0=Alu.mult)

            # --- transpose sm1 -> sm1T ---
            sm1T = ps1t.tile([128, CPAD], dt.bfloat16, tag="sm1T")
            for j in range(CT):
                ptj = psumt.tile([128, 128], dt.bfloat16, tag="pt")
                nc.tensor.transpose(ptj[:], sm1b[:, j * 128:(j + 1) * 128], id16[:])
                nc.scalar.copy(sm1T[:, j * 128:(j + 1) * 128], ptj[:])

            # --- MM3: feat_oa[m, nb] = sm1 @ centroid ---
            poa = []
            for h in range(2):
                pah = psum_f.tile([128, 512], dt.float32, tag="psf")
                for ct in range(CT):
                    nc.tensor.matmul(
                        pah[:], sm1T[:, ct * 128:(ct + 1) * 128],
                        cen[:, ct * D_BOT + h * 512: ct * D_BOT + h * 512 + 512],
                        start=(ct == 0), stop=(ct == CT - 1))
                poa.append(pah)

            # --- features_aug = feats + feat_oa;  row sum of squares ---
            fa_sb = pfa.tile([128, D_BOT], dt.float32, tag="fa")
            nc.vector.tensor_tensor(fa_sb[:, 0:512], feats_sb[:, 0:512], poa[0][:], op=Alu.add)
            nc.vector.tensor_tensor(fa_sb[:, 512:1024], feats_sb[:, 512:1024], poa[1][:], op=Alu.add)
            nc.scalar.dma_start(fa_d[rows, :], fa_sb[:])

            e2 = pe2.tile([128, CPAD], dt.float32, tag="e2")  # reused as sq scratch
            ss_lo = pst.tile([128, 1], dt.float32, tag="sql")
            ss_hi = pst.tile([128, 1], dt.float32, tag="sqh")
            nc.scalar.activation(e2[:, 0:512], fa_sb[:, 0:512], Act.Square,
                                 accum_out=ss_lo[:])
            nc.scalar.activation(e2[:, 512:1024], fa_sb[:, 512:1024], Act.Square,
                                 accum_out=ss_hi[:])
            ss = pst.tile([128, 1], dt.float32, tag="ssa")
            nc.vector.tensor_tensor(ss[:], ss_lo[:], ss_hi[:], op=Alu.add)
            nrm = pst.tile([128, 1], dt.float32, tag="nrm")
            nc.scalar.activation(nrm[:], ss[:], Act.Sqrt)
            nc.vector.tensor_scalar_max(nrm[:], nrm[:], EPS)
            rn = pst.tile([128, 1], dt.float32, tag="rn")
            nc.vector.reciprocal(rn[:], nrm[:])
            scl = pst.tile([128, 1], dt.float32, tag="scl")
            nc.vector.tensor_scalar_mul(scl[:], rn[:], 1.0 / TEMP)

            # --- outputs2 raw: featsH @ W2h.T + sm1 @ G ---
            # W2-term first for both halves (depends only on fTh, fills the
            # PE while the mask stage runs), then the G-term (needs sm1T).
            po2 = []
            for h in range(2):
                p2h = psum_o.tile([128, 512], dt.float32, tag="pso")
                for j in range(NBT):
                    nc.tensor.matmul(
                        p2h[:, :500], fTh[:, j * 128:(j + 1) * 128],
                        w2h[:, j * C + h * 500: j * C + h * 500 + 500],
                        start=(j == 0), stop=False)
                po2.append(p2h)
            for h in range(2):
                for ct in range(CT):
                    nc.tensor.matmul(
                        po2[h][:, :500], sm1T[:, ct * 128:(ct + 1) * 128],
                        g[:, ct * C + h * 500: ct * C + h * 500 + 500],
                        start=False, stop=(ct == CT - 1))

            # rowmax of raw o2 (scale-invariant since scl > 0)
            r2_lo = pst.tile([128, 1], dt.float32, tag="r2l")
            r2_hi = pst.tile([128, 1], dt.float32, tag="r2h")
            nc.vector.tensor_reduce(r2_lo[:], po2[0][:, :500], Ax.X, Alu.max)
            nc.vector.tensor_reduce(r2_hi[:], po2[1][:, :500], Ax.X, Alu.max)
            r2 = pst.tile([128, 1], dt.float32, tag="r2")
            nc.vector.tensor_tensor(r2[:], r2_lo[:], r2_hi[:], op=Alu.max)
            # bias for exp: -(r2 * scl)
            b2 = pst.tile([128, 1], dt.float32, tag="b2")
            nc.vector.scalar_tensor_tensor(b2[:], r2[:], -1.0, scl[:],
                                           op0=Alu.mult, op1=Alu.mult)
            # scale o2 out of psum into SBUF (fused scale+copy), write out
            o2sb = po2sb.tile([128, C], dt.float32, tag="o2sb")
            nc.vector.tensor_scalar(o2sb[:, 0:500], po2[0][:, :500], scl[:], None, op0=Alu.mult)
            nc.vector.tensor_scalar(o2sb[:, 500:1000], po2[1][:, :500], scl[:], None, op0=Alu.mult)
            nc.scalar.dma_start(o2_d[rows, :], o2sb[:])

            # --- softmax2 ---
            s2_lo = pst.tile([128, 1], dt.float32, tag="s2l")
            s2_hi = pst.tile([128, 1], dt.float32, tag="s2h")
            nc.scalar.activation(e2[:, 0:500], o2sb[:, 0:500], Act.Exp,
                                 bias=b2[:], scale=1.0, accum_out=s2_lo[:])
            nc.scalar.activation(e2[:, 500:1000], o2sb[:, 500:1000], Act.Exp,
                                 bias=b2[:], scale=1.0, accum_out=s2_hi[:])
            s2 = pst.tile([128, 1], dt.float32, tag="s2")
            nc.vector.tensor_tensor(s2[:], s2_lo[:], s2_hi[:], op=Alu.add)
            rs2 = pst.tile([128, 1], dt.float32, tag="rs2")
            nc.vector.reciprocal(rs2[:], s2[:])
            sm2 = psm.tile([128, C], dt.float32, tag="sm2")
            nc.vector.tensor_scalar(sm2[:, 0:500], e2[:, 0:500], rs2[:], None, op0=Alu.mult)
            nc.scalar.dma_start(sm_d[rows, 0:500], sm2[:, 0:500])
            nc.vector.tensor_scalar(sm2[:, 500:1000], e2[:, 500:1000], rs2[:], None, op0=Alu.mult)
            nc.scalar.dma_start(sm_d[rows, 500:1000], sm2[:, 500:1000])

    nc.compile()
    return nc


def _prep_inputs(features, W_b, b_b, W1, W2, centroid):
    """Host-side prep: per-core input maps with pre-transposed/split weights."""
    features = np.asarray(features, np.float32)
    W_b = np.asarray(W_b, np.float32)
    b_b = np.asarray(b_b, np.float32)
    W1 = np.asarray(W1, np.float32)
    W2 = np.asarray(W2, np.float32)
    centroid = np.asarray(centroid, np.float32)

    # W_b.T -> [p, t, nb] layout
    wbt = np.ascontiguousarray(W_b.T).reshape(KT, 128, D_BOT).transpose(1, 0, 2)
    wbh, wbl = _bf16_hl(wbt.reshape(128, KT * D_BOT))
    # W1.T -> [p, j, c]
    w1t = np.ascontiguousarray(W1.T).reshape(NBT, 128, C).transpose(1, 0, 2)
    w1h, w1l = _bf16_hl(w1t.reshape(128, NBT * C))
    # W2.T -> [p, j, c] (high part only)
    w2t = np.ascontiguousarray(W2.T).reshape(NBT, 128, C).transpose(1, 0, 2)
    w2h = w2t.reshape(128, NBT * C).astype(BF16)
    # centroid padded to 1024 rows -> [p, ct, nb]
    cpad = np.zeros((CPAD, D_BOT), np.float32)
    cpad[:C] = centroid
    cen = cpad.reshape(CT, 128, D_BOT).transpose(1, 0, 2).reshape(128, CT * D_BOT).astype(BF16)
    # G = centroid @ W2.T, padded rows -> [p, ct, c]
    G = centroid @ W2.T
    gpad = np.zeros((CPAD, C), np.float32)
    gpad[:C] = G
    g = gpad.reshape(CT, 128, C).transpose(1, 0, 2).reshape(128, CT * C).astype(BF16)
    bbh, bbl = _bf16_hl(b_b.reshape(1, D_BOT))

    shared = dict(wbh=wbh, wbl=wbl, w1h=w1h, w1l=w1l, w2h=w2h, cen=cen, g=g,
                  bbh=bbh, bbl=bbl)

    fh, fl = _bf16_hl(features)
    in_maps = []
    for c in range(NCORES):
        m = dict(shared)
        for name, arr in (("fth", fh), ("ftl", fl)):
            blk = arr[c * BC:(c + 1) * BC]                      # [1024, 2048]
            # [mt, m, t, p] -> [p, mt, t, m]
            ft = blk.reshape(MT, 128, KT, 128).transpose(3, 0, 2, 1)
            m[name] = np.ascontiguousarray(ft).reshape(128, MT * KT * 128)
        in_maps.append(m)
    return in_maps


def kernel(features, W_b, b_b, W1, W2, centroid):
    global LAST_RESULT
    from concourse.bass_utils import run_bass_kernel_spmd

    use_bias = bool(np.any(np.asarray(b_b, np.float32)))
    key = ("nc", use_bias)
    if key not in _CACHE:
        _CACHE[key] = _build(use_bias=use_bias)
    nc = _CACHE[key]

    in_maps = _prep_inputs(features, W_b, b_b, W1, W2, centroid)
    # NTFF tracing is unavailable in this container (antenv.axon_hooks is a
    # stub) and would crash run_bass_kernel_spmd if BASS_TRACE leaks in.
    os.environ["BASS_NEVER_TRACE"] = "1"
    res = run_bass_kernel_spmd(nc, in_maps, list(range(NCORES)), trace=False)
    LAST_RESULT = res

    feats = np.concatenate([r["feats"] for r in res.results], axis=0)
    fa = np.concatenate([r["fa"] for r in res.results], axis=0)
    o1 = np.concatenate([r["o1"] for r in res.results], axis=0)
    o2 = np.concatenate([r["o2"] for r in res.results], axis=0)
    sm = np.concatenate([r["sm"] for r in res.results], axis=0)
    return feats, fa, o1, o2, sm
